# revision 36
# baseline (speedup 1.0000x reference)
"""Self-contained Trainium2 Bass kernel for MultiHeadAttention (v3, bf16).

Problem: B=2, S=2048, D=1024, H=16, hd=64, with the reference's
masked_fill(mask==0, -1e-09) quirk: masked scores become ~0.0, so
exp(masked) == 1.0 in fp32 and every key position participates in the
softmax denominator. Fully-masked key blocks therefore contribute a
block-constant suffix sum of V rows (fs), added via cheap matmuls
instead of full score/attn matmuls.

All matmul numerics are bf16: fp8 anywhere in the value path costs
2-4.5% relative output error (softmax averaging shrinks the signal as
fast as the noise), beyond the 2e-2 gate.

v3 structural changes vs the original baseline:
  * Input DMA consolidated to ~16 large 3D-AP descriptors (was ~77),
    issued on sync/gpsimd so ScalarE keeps its cycles for exp.
  * v2 PSUM->SBUF copies batched: all 4 heads in one strided copy.
  * fs suffix adds batched across the 4 heads (15 TT ops, was 60).
  * Diagonal-mask affine_selects batched across the hl pair (32, was 64).
  * outproj emits one [128, 2, 512] PSUM tile per seq block: 4 matmuls,
    one DVE copy, one output DMA (was 2 copies + 2 DMAs).

Layouts (per core, all matmul operands at partition base 0):
  qt  [128, pair, S]   q^T, two heads stacked on partitions (d dims)
  ktz [128, head, S]   k^T zero-padded: even heads live on partitions
                       0-63 (64-127 zero), odd heads on 64-127 — the
                       scores matmul is then a plain K=128 matmul
                       against the pair-stacked qt.
  v2  [128, head, kj, 65]  V blocks with an appended ones column
                       (produces the softmax denominator for free).
  scores^T [sk, sq] in PSUM -> exp on ScalarE -> bf16 tiles ->
  attnU^T [65, sq] accumulated with V2 stationary, so no transposes
  are needed before the O-projection; rowsum = row 64.

Sharding: 8 cores = 2 batches x 4 head-groups (4 heads per core).
Each core computes a partial [S, D] output; the host sums the 4
partials per batch and adds bo.
"""

import numpy as np
import ml_dtypes

import concourse.bass as bass
import concourse.bacc as bacc
import concourse.tile as tile
import concourse.mybir as mybir
from concourse.bass_utils import run_bass_kernel_spmd

BF16 = mybir.dt.bfloat16
F32 = mybir.dt.float32
NPBF16 = ml_dtypes.bfloat16
AF = mybir.ActivationFunctionType

B = 2
S = 2048
D = 1024
H = 16
HD = 64
NCORES = 8
HPC = 4            # heads per core
NPAIRS = 2         # head pairs per core
NQ = S // 128      # 16 query/key blocks of 128
QCH = 512          # sq chunk width
NCH = S // QCH     # 4 chunks
KT = D // 128      # 8 contraction tiles for projections


def _emit(tc: tile.TileContext, io: dict):
    nc = tc.nc

    persist = tc.alloc_tile_pool(name="persist", bufs=1)

    # ---- constants ----
    ones128 = persist.tile([128, 128], BF16, name="ones128")
    nc.gpsimd.memset(ones128, 1.0)

    # ---- persistent SBUF arrays ----
    qt = persist.tile([128, NPAIRS, S], BF16, name="qt")
    ktz = persist.tile([128, HPC, S], BF16, name="ktz")
    v2 = persist.tile([128, HPC, NQ, 128], BF16, name="v2")
    fs = persist.tile([128, HPC, NQ, 128], BF16, name="fs")
    att = persist.tile([128, NPAIRS, S], BF16, name="att")

    qts = persist.tile([128, KT, S], BF16, name="qts")
    kts = persist.tile([128, KT, S], BF16, name="kts")
    vts = persist.tile([128, KT, S], BF16, name="vts")
    wqt = persist.tile([128, KT, 256], BF16, name="wqt")
    wkt = persist.tile([128, KT, 256], BF16, name="wkt")
    wvt = persist.tile([128, KT, 256], BF16, name="wvt")
    wot = persist.tile([128, NPAIRS, D], BF16, name="wot")
    # q/k biases as per-partition columns: bqc[:, p] = bq[p*128:(p+1)*128]
    bqc = persist.tile([128, NPAIRS], F32, name="bqc")
    bkc = persist.tile([128, NPAIRS], F32, name="bkc")

    # ---- input DMA: few large 3D-AP descriptors ----
    # gpsimd (SWDGE): small weight tensors, early; keeps ScalarE free
    nc.gpsimd.dma_start(wqt, io["WqT"].rearrange("(t p) m -> p t m", p=128))
    nc.gpsimd.dma_start(wkt, io["WkT"].rearrange("(t p) m -> p t m", p=128))
    nc.gpsimd.dma_start(wvt, io["WvT"].rearrange("(t p) m -> p t m", p=128))
    nc.gpsimd.dma_start(wot, io["WoT"].rearrange("(o p) m -> p o m", p=128))
    nc.scalar.dma_start(bqc, io["bqc"])
    nc.scalar.dma_start(bkc, io["bkc"])

    # sync (HWDGE): activations; chunk 0 first so projections start early
    def xt_src(t_io, c0, c1):
        return t_io.rearrange("(t p) s -> p t s", p=128)[:, :, c0:c1]

    dma = nc.sync
    # All big transfers on the sync HWDGE ring (issuing a large DMA can
    # block the issuing engine for ~10us -> never on ScalarE).  The 16
    # SDMA engines round-robin across ALL outstanding DMAs, so the
    # first-needed bytes only arrive quickly if later bulk transfers
    # are STAGED: bursts 2/3 get dependency edges onto early matmuls
    # (below) so their descriptors enter the ring later.
    dma.dma_start(qts[:, 0:4, 0:QCH], xt_src(io["QT"], 0, QCH)[:, 0:4, :])
    dma.dma_start(qts[:, 4:8, 0:QCH], xt_src(io["QT"], 0, QCH)[:, 4:8, :])
    dma.dma_start(kts[:, 0:4, 0:QCH], xt_src(io["KT"], 0, QCH)[:, 0:4, :])
    dma.dma_start(kts[:, 4:8, 0:QCH], xt_src(io["KT"], 0, QCH)[:, 4:8, :])
    dma.dma_start(vts[:, :, 0:QCH], xt_src(io["VT"], 0, QCH))
    dma.dma_start(vts[:, :, QCH:2 * QCH], xt_src(io["VT"], QCH, 2 * QCH))
    dma.dma_start(qts[:, :, QCH:2 * QCH], xt_src(io["QT"], QCH, 2 * QCH))
    dma.dma_start(kts[:, :, QCH:2 * QCH], xt_src(io["KT"], QCH, 2 * QCH))
    dma.dma_start(vts[:, :, 2 * QCH:], xt_src(io["VT"], 2 * QCH, S))
    dma.dma_start(qts[:, :, 2 * QCH:], xt_src(io["QT"], 2 * QCH, S))
    dma.dma_start(kts[:, :, 2 * QCH:], xt_src(io["KT"], 2 * QCH, S))

    nc.gpsimd.memset(v2[:, :, :, 64:128], 1.0)  # 64 ones columns -> Z on rows 64-127

    pb_s = tc.alloc_tile_pool(name="pb_scores", bufs=2, space="PSUM")
    pb_a = tc.alloc_tile_pool(name="pb_attnu", bufs=2, space="PSUM")
    pb_e = tc.alloc_tile_pool(name="pb_exp", bufs=7)
    pb_o = tc.alloc_tile_pool(name="pb_ob", bufs=3)
    pb_r = tc.alloc_tile_pool(name="pb_recip", bufs=2)

    def vproj_unit(st):
        if True:
            psv_t = pb_s.tile([128, 2, QCH], F32, tag="sps", name=f"ps_v{st}")
            ps_v = psv_t[:, 0, 0:256]
            for t in range(KT):
                nc.tensor.matmul(ps_v, vts[:, t, st * 128:(st + 1) * 128],
                                 wvt[:, t, :], start=(t == 0),
                                 stop=(t == KT - 1))
            # bv is NOT added here: it passes through the softmax average
            # exactly (weights sum to 1), so the host folds bv @ Wo.T into
            # the bo add instead.
            # one strided copy: all 4 heads at once
            nc.vector.tensor_copy(v2[:, :, st, 0:64],
                                  ps_v.rearrange("p (h d) -> p h d", h=4))

    def qproj_unit(c, p):
        sq = slice(c * QCH, (c + 1) * QCH)
        psq_t = pb_s.tile([128, 2, QCH], F32, tag="sps", name=f"ps_q{p}_{c}")
        ps_q = psq_t[:, 0, :]
        for t in range(KT):
            nc.tensor.matmul(ps_q, wqt[:, t, p * 128:(p + 1) * 128],
                             qts[:, t, sq], start=(t == 0),
                             stop=(t == KT - 1))
        # bias folded into the PSUM->SBUF copy (per-partition scalar add)
        nc.vector.tensor_scalar_add(qt[:, p, sq], ps_q, bqc[:, p:p + 1])

    def kproj_unit(c, p):
        sq = slice(c * QCH, (c + 1) * QCH)
        psk_t = pb_s.tile([128, 2, QCH], F32, tag="sps", name=f"ps_k{p}_{c}")
        ps_k = psk_t[:, 0, :]
        for t in range(KT):
            nc.tensor.matmul(ps_k, wkt[:, t, p * 128:(p + 1) * 128],
                             kts[:, t, sq], start=(t == 0),
                             stop=(t == KT - 1))
        nc.vector.tensor_scalar_add(ktz[0:64, 2 * p, sq], ps_k[0:64, :],
                                    bkc[0:64, p:p + 1])
        nc.vector.tensor_scalar_add(ktz[64:128, 2 * p + 1, sq],
                                    ps_k[64:128, :], bkc[64:128, p:p + 1])

    def qkproj(c):
        for p in range(NPAIRS):
            qproj_unit(c, p)
            kproj_unit(c, p)

    def folded_suffixes():
        nc.vector.memset(fs[:, :, NQ - 1, :], 0.0)
        for q in range(NQ - 2, -1, -1):
            # all 4 heads in one strided TT add
            nc.vector.tensor_add(fs[:, :, q, :], fs[:, :, q + 1, :],
                                 v2[:, :, q + 1, :])

    aups_tiles = {}

    def chunk_loop(c, fillers=()):
        """scores -> exp -> attnU^T accumulation for chunk c, both pairs.

        Software-pipelined LEAD score units ahead of attnU (the PE queue
        is strict FIFO, so attnU's exp-wait would otherwise serialize
        every kj).  One filler work-unit (an independent PE job: v/q/k
        projection unit, outproj unit, fs chain step) is emitted per kj
        iteration so ScalarE's exp stream never starves while pure-PE
        phases run.
        """
        fillers = list(fillers)
        nkj = 4 * c + 4
        exts = {}
        LEAD = 3

        def scores_unit(p, kj):
            c0 = max(kj - 4 * c, 0) * 128   # first valid col in chunk
            sps = pb_s.tile([128, 2, QCH], F32, tag="sps",
                            name=f"sps{p}_{c}_{kj}")
            for hl in range(2):
                # K=64 on row-group hl*64: the two heads' score matmuls
                # occupy disjoint 64-row strips of the PE array and run
                # concurrently (tile_position derived from the operand
                # partition bases)
                h64 = slice(hl * 64, hl * 64 + 64)
                nc.tensor.matmul(
                    sps[:, hl, c0:QCH],
                    ktz[h64, 2 * p + hl, kj * 128:(kj + 1) * 128],
                    qt[h64, p, c * QCH + c0:(c + 1) * QCH],
                    start=True, stop=True)
            ext = pb_e.tile([128, 2, QCH], BF16, tag="ext",
                            name=f"ext{p}_{c}_{kj}")
            nc.scalar.activation(ext[:, :, c0:QCH], sps[:, :, c0:QCH],
                                 AF.Exp, scale=0.125)
            if kj >= 4 * c:  # diagonal block: masked exp entries -> 1.0
                nc.gpsimd.affine_select(
                    out=ext[:, :, c0:c0 + 128],
                    in_=ext[:, :, c0:c0 + 128],
                    compare_op=mybir.AluOpType.is_ge,
                    fill=1.0, base=0,
                    pattern=[[0, 2], [1, 128]], channel_multiplier=-1)
            exts[(p, kj)] = ext

        def attnu_unit(p, kj, aups):
            c0 = max(kj - 4 * c, 0) * 128
            ext = exts.pop((p, kj))
            for hl in range(2):
                # masked cols < c0 get their (block-constant)
                # contribution from the FS matmuls
                nc.tensor.matmul(
                    aups[:, hl, c0:QCH],
                    v2[:, 2 * p + hl, kj, :],
                    ext[:, hl, c0:QCH],
                    start=(kj == 0),
                    stop=(kj == nkj - 1 and c > 0))

        for p in range(NPAIRS):
            aups = pb_a.tile([128, 2, QCH], F32, tag="aups", name=f"aups{p}_{c}")
            aups_tiles[(p, c)] = aups
            for kj in range(min(LEAD, nkj)):
                scores_unit(p, kj)
            for kj in range(nkj):
                if kj + LEAD < nkj:
                    scores_unit(p, kj + LEAD)
                attnu_unit(p, kj, aups)
                if c > 0 and 1 <= kj <= 4:
                    # fs suffix adds, spread one per kj, in the order the
                    # fs chain produces them (high qi first); they commute
                    # with the accumulation
                    ql = 4 - kj
                    qi = 4 * c + ql
                    if qi < NQ - 1:
                        for hl in range(2):
                            nc.tensor.matmul(
                                aups[:, hl, ql * 128:(ql + 1) * 128],
                                fs[:, 2 * p + hl, qi, :], ones128,
                                start=False, stop=False)
                slots_left = (NPAIRS - 1 - p) * nkj + (nkj - 1 - kj)
                want = len(fillers) - slots_left  # drain evenly
                for _ in range(max(1 if fillers else 0, want)):
                    if fillers:
                        fillers.pop(0)()
            if c > 0:
                # finalize this pair immediately: hides the recip chain
                # inside the chunk instead of stalling the next chunk's
                # attnU on the aups buffer WAR
                finalize_pair(p, c)
        for f in fillers:
            f()

    def finalize_pair(p, c):
        """Rowsum reciprocal + normalize into att for (pair, chunk)."""
        ch = slice(c * QCH, (c + 1) * QCH)
        aups = aups_tiles[(p, c)]
        lnr = pb_r.tile([128, 2 * QCH], F32, tag="lr", name=f"lnr{p}_{c}")
        nc.scalar.activation(lnr[64:128, :], aups[64:128, :, :], AF.Ln)
        nc.scalar.activation(lnr[64:128, :], lnr[64:128, :], AF.Exp,
                             scale=-1.0)  # in-place: lnr becomes 1/Z
        for hl in range(2):
            nc.vector.tensor_mul(
                att[hl * 64:(hl + 1) * 64, p, ch],
                aups[0:64, hl, :],
                lnr[64:128, hl * QCH:(hl + 1) * QCH])

    def finalize0():
        """Chunk 0 only: late FS adds, then per-pair finalize."""
        for p in range(NPAIRS):
            aups = aups_tiles[(p, 0)]
            for hl in range(2):
                for ql in range(4):
                    nc.tensor.matmul(
                        aups[:, hl, ql * 128:(ql + 1) * 128],
                        fs[:, 2 * p + hl, ql, :], ones128,
                        start=False, stop=(ql == 3))
            finalize_pair(p, 0)

    def outproj_unit(st):
        pso = pb_s.tile([128, 2, QCH], F32, tag="sps", name=f"pso{st}")
        for dc in range(2):
            for p in range(NPAIRS):
                nc.tensor.matmul(
                    pso[:, dc, :],
                    att[:, p, st * 128:(st + 1) * 128],
                    wot[:, p, dc * 512:(dc + 1) * 512],
                    start=(p == 0), stop=(p == NPAIRS - 1))
        ob = pb_o.tile([128, 2, QCH], BF16, tag="ob", name=f"ob{st}")
        nc.vector.tensor_copy(ob, pso)
        dma.dma_start(
            io["out"][st * 128:(st + 1) * 128, :].rearrange(
                "s (a m) -> s a m", a=2), ob)

    def vproj_filler(st):
        """One V-projection block + the fs chain step it unblocks."""
        def f():
            vproj_unit(st)
            q = st - 1
            if q >= 4:
                fs_step(q)
            elif st == 4:
                for qq in range(3, -1, -1):
                    fs_step(qq)
        return f

    def fs_step(q):
        # all 4 heads in one strided TT add
        nc.vector.tensor_add(fs[:, :, q, :], fs[:, :, q + 1, :],
                             v2[:, :, q + 1, :])
    # Emission order == per-engine FIFO order (the scheduler follows
    # priorities).  Chunk 0's fillers run the remaining V projections in
    # DESCENDING block order with the fs suffix-chain steps interleaved
    # (the chain consumes blocks high-to-low), then the chunk-1 q/k
    # projections, so finalize0 is ready the moment chunk 0 drains.
    nc.vector.memset(fs[:, :, NQ - 1, :], 0.0)
    qkproj(0)
    for st in range(4):
        vproj_unit(st)
    c0_fillers = [vproj_filler(st) for st in range(15, 3, -1)]
    c0_fillers += [lambda: qproj_unit(1, 0), lambda: kproj_unit(1, 0),
                   lambda: qproj_unit(1, 1), lambda: kproj_unit(1, 1)]
    chunk_loop(0, c0_fillers)
    finalize0()
    c1_fillers = [lambda p=p: qproj_unit(2, p) for p in range(2)]
    c1_fillers += [lambda p=p: kproj_unit(2, p) for p in range(2)]
    c1_fillers += [lambda st=st: outproj_unit(st) for st in range(0, 4)]
    chunk_loop(1, c1_fillers)
    c2_fillers = [lambda p=p: qproj_unit(3, p) for p in range(2)]
    c2_fillers += [lambda p=p: kproj_unit(3, p) for p in range(2)]
    c2_fillers += [lambda st=st: outproj_unit(st) for st in range(4, 8)]
    chunk_loop(2, c2_fillers)
    c3_fillers = [lambda st=st: outproj_unit(st) for st in range(8, 12)]
    chunk_loop(3, c3_fillers)
    for st in range(12, 16):
        outproj_unit(st)

    pb_r.release()
    pb_o.release()
    pb_e.release()
    pb_a.release()
    pb_s.release()
    persist.release()


_CACHED = None


def _patch_act_tables():
    """Make Exp and Ln resolve to the single combined table set so the
    per-chunk recip (Ln/Exp) doesn't thrash ACT_TABLE_LOADs against the
    softmax Exp calls."""
    from concourse import hw_specs
    orig = hw_specs.get_activation_tables

    def patched(arch):
        t = dict(orig(arch))
        if "natural_log_exp_and_others" in t:
            for name in t:
                if name != "natural_log_exp_and_others":
                    t[name] = t[name] - {AF.Exp, AF.Ln}
        return t

    bacc.get_activation_tables = patched


def _build():
    global _CACHED
    if _CACHED is not None:
        return _CACHED
    _patch_act_tables()
    nc = bacc.Bacc("TRN2", target_bir_lowering=False, debug=False)
    io = {
        "QT": nc.dram_tensor("QT", [D, S], BF16, kind="ExternalInput").ap(),
        "KT": nc.dram_tensor("KT", [D, S], BF16, kind="ExternalInput").ap(),
        "VT": nc.dram_tensor("VT", [D, S], BF16, kind="ExternalInput").ap(),
        "WqT": nc.dram_tensor("WqT", [D, 256], BF16, kind="ExternalInput").ap(),
        "WkT": nc.dram_tensor("WkT", [D, 256], BF16, kind="ExternalInput").ap(),
        "WvT": nc.dram_tensor("WvT", [D, 256], BF16, kind="ExternalInput").ap(),
        "WoT": nc.dram_tensor("WoT", [256, D], BF16, kind="ExternalInput").ap(),
        "bqc": nc.dram_tensor("bqc", [128, NPAIRS], F32,
                              kind="ExternalInput").ap(),
        "bkc": nc.dram_tensor("bkc", [128, NPAIRS], F32,
                              kind="ExternalInput").ap(),
        "out": nc.dram_tensor("out", [S, D], BF16, kind="ExternalOutput").ap(),
    }
    with tile.TileContext(nc) as tc:
        _emit(tc, io)
    nc.compile()
    _CACHED = (nc, io)
    return _CACHED


def make_in_maps(Q, K, V, Wq, bq, Wk, bk, Wv, bv, Wo):
    """Build the 8 per-core input dicts (host-side sharding)."""
    Q = np.asarray(Q, np.float32)
    K = np.asarray(K, np.float32)
    V = np.asarray(V, np.float32)
    qt = [np.ascontiguousarray(Q[b].T).astype(NPBF16) for b in range(B)]
    kt = [np.ascontiguousarray(K[b].T).astype(NPBF16) for b in range(B)]
    vt = [np.ascontiguousarray(V[b].T).astype(NPBF16) for b in range(B)]
    in_maps = []
    for core in range(NCORES):
        b, g = divmod(core, 4)
        rows = slice(g * 256, (g + 1) * 256)
        in_maps.append({
            "QT": qt[b], "KT": kt[b], "VT": vt[b],
            "WqT": np.ascontiguousarray(
                np.asarray(Wq, np.float32)[rows].T).astype(NPBF16),
            "WkT": np.ascontiguousarray(
                np.asarray(Wk, np.float32)[rows].T).astype(NPBF16),
            "WvT": np.ascontiguousarray(
                np.asarray(Wv, np.float32)[rows].T).astype(NPBF16),
            "WoT": np.ascontiguousarray(
                np.asarray(Wo, np.float32)[:, rows].T).astype(NPBF16),
            "bqc": np.ascontiguousarray(
                np.asarray(bq, np.float32)[rows].reshape(2, 128).T),
            "bkc": np.ascontiguousarray(
                np.asarray(bk, np.float32)[rows].reshape(2, 128).T),
        })
    return in_maps


def kernel(Q, K, V, mask, Wq, bq, Wk, bk, Wv, bv, Wo, bo, _results_hook=None):
    nc, _io = _build()
    in_maps = make_in_maps(Q, K, V, Wq, bq, Wk, bk, Wv, bv, Wo)
    res = run_bass_kernel_spmd(nc, in_maps, core_ids=list(range(NCORES)))
    if _results_hook is not None:
        _results_hook(res)
    out = np.zeros((B, S, D), np.float32)
    for core in range(NCORES):
        out[core // 4] += np.asarray(res.results[core]["out"], np.float32)
    # bv passes through the softmax average exactly; its output-space
    # contribution is the constant row bv @ Wo.T, folded in here.
    out += np.asarray(bo, np.float32) + (
        np.asarray(bv, np.float32) @ np.asarray(Wo, np.float32).T)
    return out


# revision 39
# speedup vs baseline: 1.0226x; 1.0226x over previous
"""Self-contained Trainium2 Bass kernel for MultiHeadAttention (v3, bf16).

Problem: B=2, S=2048, D=1024, H=16, hd=64, with the reference's
masked_fill(mask==0, -1e-09) quirk: masked scores become ~0.0, so
exp(masked) == 1.0 in fp32 and every key position participates in the
softmax denominator. Fully-masked key blocks therefore contribute a
block-constant suffix sum of V rows (fs), added via cheap matmuls
instead of full score/attn matmuls.

All matmul numerics are bf16: fp8 anywhere in the value path costs
2-4.5% relative output error (softmax averaging shrinks the signal as
fast as the noise), beyond the 2e-2 gate.

v3 structural changes vs the original baseline:
  * Input DMA consolidated to ~16 large 3D-AP descriptors (was ~77),
    issued on sync/gpsimd so ScalarE keeps its cycles for exp.
  * v2 PSUM->SBUF copies batched: all 4 heads in one strided copy.
  * fs suffix adds batched across the 4 heads (15 TT ops, was 60).
  * Diagonal-mask affine_selects batched across the hl pair (32, was 64).
  * outproj emits one [128, 2, 512] PSUM tile per seq block: 4 matmuls,
    one DVE copy, one output DMA (was 2 copies + 2 DMAs).

Layouts (per core, all matmul operands at partition base 0):
  qt  [128, pair, S]   q^T, two heads stacked on partitions (d dims)
  ktz [128, head, S]   k^T zero-padded: even heads live on partitions
                       0-63 (64-127 zero), odd heads on 64-127 — the
                       scores matmul is then a plain K=128 matmul
                       against the pair-stacked qt.
  v2  [128, head, kj, 65]  V blocks with an appended ones column
                       (produces the softmax denominator for free).
  scores^T [sk, sq] in PSUM -> exp on ScalarE -> bf16 tiles ->
  attnU^T [65, sq] accumulated with V2 stationary, so no transposes
  are needed before the O-projection; rowsum = row 64.

Sharding: 8 cores = 2 batches x 4 head-groups (4 heads per core).
Each core computes a partial [S, D] output; the host sums the 4
partials per batch and adds bo.
"""

import numpy as np
import ml_dtypes

import concourse.bass as bass
import concourse.bacc as bacc
import concourse.tile as tile
import concourse.mybir as mybir
from concourse.bass_utils import run_bass_kernel_spmd

BF16 = mybir.dt.bfloat16
F32 = mybir.dt.float32
NPBF16 = ml_dtypes.bfloat16
AF = mybir.ActivationFunctionType

B = 2
S = 2048
D = 1024
H = 16
HD = 64
NCORES = 8
HPC = 4            # heads per core
NPAIRS = 2         # head pairs per core
NQ = S // 128      # 16 query/key blocks of 128
QCH = 512          # sq chunk width
NCH = S // QCH     # 4 chunks
KT = D // 128      # 8 contraction tiles for projections


def _emit(tc: tile.TileContext, io: dict):
    nc = tc.nc

    persist = tc.alloc_tile_pool(name="persist", bufs=1)

    # ---- constants ----
    ones128 = persist.tile([128, 128], BF16, name="ones128")
    nc.gpsimd.memset(ones128, 1.0)

    # ---- persistent SBUF arrays ----
    qt = persist.tile([128, NPAIRS, S], BF16, name="qt")
    ktz = persist.tile([128, HPC, S], BF16, name="ktz")
    v2 = persist.tile([128, HPC, NQ, 128], BF16, name="v2")
    fs = persist.tile([128, HPC, NQ, 128], BF16, name="fs")
    att = persist.tile([128, NPAIRS, S], BF16, name="att")

    qts = persist.tile([128, KT, S], BF16, name="qts")
    kts = persist.tile([128, KT, S], BF16, name="kts")
    vts = persist.tile([128, KT, S], BF16, name="vts")
    wqt = persist.tile([128, KT, 256], BF16, name="wqt")
    wkt = persist.tile([128, KT, 256], BF16, name="wkt")
    wvt = persist.tile([128, KT, 256], BF16, name="wvt")
    wot = persist.tile([128, NPAIRS, D], BF16, name="wot")
    # q/k biases as per-partition columns: bqc[:, p] = bq[p*128:(p+1)*128]
    bqc = persist.tile([128, NPAIRS], F32, name="bqc")
    bkc = persist.tile([128, NPAIRS], F32, name="bkc")

    # ---- input DMA: few large 3D-AP descriptors ----
    # gpsimd (SWDGE): small weight tensors, early; keeps ScalarE free
    nc.gpsimd.dma_start(wqt, io["WqT"].rearrange("(t p) m -> p t m", p=128))
    nc.gpsimd.dma_start(wkt, io["WkT"].rearrange("(t p) m -> p t m", p=128))
    nc.gpsimd.dma_start(wvt, io["WvT"].rearrange("(t p) m -> p t m", p=128))
    nc.gpsimd.dma_start(wot, io["WoT"].rearrange("(o p) m -> p o m", p=128))
    nc.scalar.dma_start(bqc, io["bqc"])
    nc.scalar.dma_start(bkc, io["bkc"])

    # sync (HWDGE): activations; chunk 0 first so projections start early
    def xt_src(t_io, c0, c1):
        return t_io.rearrange("(t p) s -> p t s", p=128)[:, :, c0:c1]

    dma = nc.sync
    # All big transfers on the sync HWDGE ring (issuing a large DMA can
    # block the issuing engine for ~10us -> never on ScalarE).  The 16
    # SDMA engines round-robin across ALL outstanding DMAs, so the
    # first-needed bytes only arrive quickly if later bulk transfers
    # are STAGED: bursts 2/3 get dependency edges onto early matmuls
    # (below) so their descriptors enter the ring later.
    dma.dma_start(qts[:, 0:4, 0:QCH], xt_src(io["QT"], 0, QCH)[:, 0:4, :])
    dma.dma_start(qts[:, 4:8, 0:QCH], xt_src(io["QT"], 0, QCH)[:, 4:8, :])
    dma.dma_start(kts[:, 0:4, 0:QCH], xt_src(io["KT"], 0, QCH)[:, 0:4, :])
    dma.dma_start(kts[:, 4:8, 0:QCH], xt_src(io["KT"], 0, QCH)[:, 4:8, :])
    dma.dma_start(vts[:, :, 0:QCH], xt_src(io["VT"], 0, QCH))
    dma.dma_start(vts[:, :, QCH:2 * QCH], xt_src(io["VT"], QCH, 2 * QCH))
    dma.dma_start(qts[:, :, QCH:2 * QCH], xt_src(io["QT"], QCH, 2 * QCH))
    dma.dma_start(kts[:, :, QCH:2 * QCH], xt_src(io["KT"], QCH, 2 * QCH))
    dma.dma_start(vts[:, :, 2 * QCH:], xt_src(io["VT"], 2 * QCH, S))
    dma.dma_start(qts[:, :, 2 * QCH:], xt_src(io["QT"], 2 * QCH, S))
    dma.dma_start(kts[:, :, 2 * QCH:], xt_src(io["KT"], 2 * QCH, S))

    nc.gpsimd.memset(v2[:, :, :, 64:128], 1.0)  # 64 ones columns -> Z on rows 64-127

    pb_s = tc.alloc_tile_pool(name="pb_scores", bufs=2, space="PSUM")
    pb_a = tc.alloc_tile_pool(name="pb_attnu", bufs=2, space="PSUM")
    pb_e = tc.alloc_tile_pool(name="pb_exp", bufs=9)
    pb_o = tc.alloc_tile_pool(name="pb_ob", bufs=2)
    pb_r = tc.alloc_tile_pool(name="pb_recip", bufs=2)

    def vproj_unit(st):
        if True:
            psv_t = pb_s.tile([128, 2, QCH], F32, tag="sps", name=f"ps_v{st}")
            ps_v = psv_t[:, 0, 0:256]
            for t in range(KT):
                nc.tensor.matmul(ps_v, vts[:, t, st * 128:(st + 1) * 128],
                                 wvt[:, t, :], start=(t == 0),
                                 stop=(t == KT - 1))
            # bv is NOT added here: it passes through the softmax average
            # exactly (weights sum to 1), so the host folds bv @ Wo.T into
            # the bo add instead.
            # one strided copy: all 4 heads at once
            nc.vector.tensor_copy(v2[:, :, st, 0:64],
                                  ps_v.rearrange("p (h d) -> p h d", h=4))

    def qproj_unit(c, p):
        sq = slice(c * QCH, (c + 1) * QCH)
        psq_t = pb_s.tile([128, 2, QCH], F32, tag="sps", name=f"ps_q{p}_{c}")
        ps_q = psq_t[:, 0, :]
        for t in range(KT):
            nc.tensor.matmul(ps_q, wqt[:, t, p * 128:(p + 1) * 128],
                             qts[:, t, sq], start=(t == 0),
                             stop=(t == KT - 1))
        # bias folded into the PSUM->SBUF copy (per-partition scalar add)
        nc.vector.tensor_scalar_add(qt[:, p, sq], ps_q, bqc[:, p:p + 1])

    def kproj_unit(c, p):
        sq = slice(c * QCH, (c + 1) * QCH)
        psk_t = pb_s.tile([128, 2, QCH], F32, tag="sps", name=f"ps_k{p}_{c}")
        ps_k = psk_t[:, 0, :]
        for t in range(KT):
            nc.tensor.matmul(ps_k, wkt[:, t, p * 128:(p + 1) * 128],
                             kts[:, t, sq], start=(t == 0),
                             stop=(t == KT - 1))
        nc.vector.tensor_scalar_add(ktz[0:64, 2 * p, sq], ps_k[0:64, :],
                                    bkc[0:64, p:p + 1])
        nc.vector.tensor_scalar_add(ktz[64:128, 2 * p + 1, sq],
                                    ps_k[64:128, :], bkc[64:128, p:p + 1])

    def qkproj(c):
        for p in range(NPAIRS):
            qproj_unit(c, p)
            kproj_unit(c, p)

    def folded_suffixes():
        nc.vector.memset(fs[:, :, NQ - 1, :], 0.0)
        for q in range(NQ - 2, -1, -1):
            # all 4 heads in one strided TT add
            nc.vector.tensor_add(fs[:, :, q, :], fs[:, :, q + 1, :],
                                 v2[:, :, q + 1, :])

    aups_tiles = {}

    def chunk_loop(c, fillers=()):
        """scores -> exp -> attnU^T accumulation for chunk c, both pairs.

        Software-pipelined LEAD score units ahead of attnU (the PE queue
        is strict FIFO, so attnU's exp-wait would otherwise serialize
        every kj).  One filler work-unit (an independent PE job: v/q/k
        projection unit, outproj unit, fs chain step) is emitted per kj
        iteration so ScalarE's exp stream never starves while pure-PE
        phases run.
        """
        fillers = list(fillers)
        nkj = 4 * c + 4
        exts = {}
        LEAD = 3

        def scores_unit(p, kj):
            c0 = max(kj - 4 * c, 0) * 128   # first valid col in chunk
            sps = pb_s.tile([128, 2, QCH], F32, tag="sps",
                            name=f"sps{p}_{c}_{kj}")
            for hl in range(2):
                # K=64 on row-group hl*64: the two heads' score matmuls
                # occupy disjoint 64-row strips of the PE array and run
                # concurrently (tile_position derived from the operand
                # partition bases)
                h64 = slice(hl * 64, hl * 64 + 64)
                nc.tensor.matmul(
                    sps[:, hl, c0:QCH],
                    ktz[h64, 2 * p + hl, kj * 128:(kj + 1) * 128],
                    qt[h64, p, c * QCH + c0:(c + 1) * QCH],
                    start=True, stop=True)
            ext = pb_e.tile([128, 2, QCH], BF16, tag="ext",
                            name=f"ext{p}_{c}_{kj}")
            nc.scalar.activation(ext[:, :, c0:QCH], sps[:, :, c0:QCH],
                                 AF.Exp, scale=0.125)
            if kj >= 4 * c:  # diagonal block: masked exp entries -> 1.0
                nc.gpsimd.affine_select(
                    out=ext[:, :, c0:c0 + 128],
                    in_=ext[:, :, c0:c0 + 128],
                    compare_op=mybir.AluOpType.is_ge,
                    fill=1.0, base=0,
                    pattern=[[0, 2], [1, 128]], channel_multiplier=-1)
            exts[(p, kj)] = ext

        def attnu_unit(p, kj, aups):
            c0 = max(kj - 4 * c, 0) * 128
            ext = exts.pop((p, kj))
            for hl in range(2):
                # masked cols < c0 get their (block-constant)
                # contribution from the FS matmuls
                nc.tensor.matmul(
                    aups[:, hl, c0:QCH],
                    v2[:, 2 * p + hl, kj, :],
                    ext[:, hl, c0:QCH],
                    start=(kj == 0),
                    stop=(kj == nkj - 1 and c > 0))

        aups_t = {}
        for p in range(NPAIRS):
            aups_t[p] = pb_a.tile([128, 2, QCH], F32, tag="aups",
                                  name=f"aups{p}_{c}")
            aups_tiles[(p, c)] = aups_t[p]
            scores_unit(p, 0)
        # pairs interleaved: two independent score->exp->attnU chains keep
        # ScalarE fed when either chain waits on a cross-engine hazard
        for kj in range(nkj):
            if kj + 1 < nkj:
                for p in range(NPAIRS):
                    scores_unit(p, kj + 1)
            for p in range(NPAIRS):
                attnu_unit(p, kj, aups_t[p])
                if c > 0 and 1 <= kj <= 4:
                    # fs suffix adds, spread one per kj, in the order the
                    # fs chain produces them (high qi first); they commute
                    # with the accumulation
                    ql = 4 - kj
                    qi = 4 * c + ql
                    if qi < NQ - 1:
                        for hl in range(2):
                            nc.tensor.matmul(
                                aups_t[p][:, hl, ql * 128:(ql + 1) * 128],
                                fs[:, 2 * p + hl, qi, :], ones128,
                                start=False, stop=False)
                slots_left = (nkj - 1 - kj) * NPAIRS + (NPAIRS - 1 - p)
                want = len(fillers) - slots_left  # drain evenly
                for _ in range(max(1 if fillers else 0, want)):
                    if fillers:
                        fillers.pop(0)()
        if c > 0:
            # finalize both pairs now: hides the recip chain inside the
            # chunk instead of stalling the next chunk's attnU on the
            # aups buffer WAR
            for p in range(NPAIRS):
                finalize_pair(p, c)
        for f in fillers:
            f()

    def finalize_pair(p, c):
        """Rowsum reciprocal + normalize into att for (pair, chunk)."""
        ch = slice(c * QCH, (c + 1) * QCH)
        aups = aups_tiles[(p, c)]
        lnr = pb_r.tile([128, 2 * QCH], F32, tag="lr", name=f"lnr{p}_{c}")
        nc.scalar.activation(lnr[64:128, :], aups[64:128, :, :], AF.Ln)
        nc.scalar.activation(lnr[64:128, :], lnr[64:128, :], AF.Exp,
                             scale=-1.0)  # in-place: lnr becomes 1/Z
        for hl in range(2):
            nc.vector.tensor_mul(
                att[hl * 64:(hl + 1) * 64, p, ch],
                aups[0:64, hl, :],
                lnr[64:128, hl * QCH:(hl + 1) * QCH])

    def finalize0():
        """Chunk 0 only: late FS adds, then per-pair finalize."""
        for p in range(NPAIRS):
            aups = aups_tiles[(p, 0)]
            for hl in range(2):
                for ql in range(4):
                    nc.tensor.matmul(
                        aups[:, hl, ql * 128:(ql + 1) * 128],
                        fs[:, 2 * p + hl, ql, :], ones128,
                        start=False, stop=(ql == 3))
            finalize_pair(p, 0)

    def outproj_unit(st):
        pso = pb_s.tile([128, 2, QCH], F32, tag="sps", name=f"pso{st}")
        for dc in range(2):
            for p in range(NPAIRS):
                nc.tensor.matmul(
                    pso[:, dc, :],
                    att[:, p, st * 128:(st + 1) * 128],
                    wot[:, p, dc * 512:(dc + 1) * 512],
                    start=(p == 0), stop=(p == NPAIRS - 1))
        ob = pb_o.tile([128, 2, QCH], BF16, tag="ob", name=f"ob{st}")
        nc.vector.tensor_copy(ob, pso)
        dma.dma_start(
            io["out"][st * 128:(st + 1) * 128, :].rearrange(
                "s (a m) -> s a m", a=2), ob)

    def vproj_filler(st):
        """One V-projection block + the fs chain step it unblocks."""
        def f():
            vproj_unit(st)
            q = st - 1
            if q >= 4:
                fs_step(q)
            elif st == 4:
                for qq in range(3, -1, -1):
                    fs_step(qq)
        return f

    def fs_step(q):
        # all 4 heads in one strided TT add
        nc.vector.tensor_add(fs[:, :, q, :], fs[:, :, q + 1, :],
                             v2[:, :, q + 1, :])
    # Emission order == per-engine FIFO order (the scheduler follows
    # priorities).  Chunk 0's fillers run the remaining V projections in
    # DESCENDING block order with the fs suffix-chain steps interleaved
    # (the chain consumes blocks high-to-low), then the chunk-1 q/k
    # projections, so finalize0 is ready the moment chunk 0 drains.
    nc.vector.memset(fs[:, :, NQ - 1, :], 0.0)
    qkproj(0)
    for st in range(4):
        vproj_unit(st)
    c0_fillers = [vproj_filler(st) for st in range(15, 3, -1)]
    c0_fillers += [lambda: qproj_unit(1, 0), lambda: kproj_unit(1, 0),
                   lambda: qproj_unit(1, 1), lambda: kproj_unit(1, 1)]
    chunk_loop(0, c0_fillers)
    finalize0()
    c1_fillers = [lambda p=p: qproj_unit(2, p) for p in range(2)]
    c1_fillers += [lambda p=p: kproj_unit(2, p) for p in range(2)]
    c1_fillers += [lambda st=st: outproj_unit(st) for st in range(0, 4)]
    chunk_loop(1, c1_fillers)
    c2_fillers = [lambda p=p: qproj_unit(3, p) for p in range(2)]
    c2_fillers += [lambda p=p: kproj_unit(3, p) for p in range(2)]
    c2_fillers += [lambda st=st: outproj_unit(st) for st in range(4, 8)]
    chunk_loop(2, c2_fillers)
    c3_fillers = [lambda st=st: outproj_unit(st) for st in range(8, 12)]
    chunk_loop(3, c3_fillers)
    for st in range(12, 16):
        outproj_unit(st)

    pb_r.release()
    pb_o.release()
    pb_e.release()
    pb_a.release()
    pb_s.release()
    persist.release()


_CACHED = None


def _patch_act_tables():
    """Make Exp and Ln resolve to the single combined table set so the
    per-chunk recip (Ln/Exp) doesn't thrash ACT_TABLE_LOADs against the
    softmax Exp calls."""
    from concourse import hw_specs
    orig = hw_specs.get_activation_tables

    def patched(arch):
        t = dict(orig(arch))
        if "natural_log_exp_and_others" in t:
            for name in t:
                if name != "natural_log_exp_and_others":
                    t[name] = t[name] - {AF.Exp, AF.Ln}
        return t

    bacc.get_activation_tables = patched


def _build():
    global _CACHED
    if _CACHED is not None:
        return _CACHED
    _patch_act_tables()
    nc = bacc.Bacc("TRN2", target_bir_lowering=False, debug=False)
    io = {
        "QT": nc.dram_tensor("QT", [D, S], BF16, kind="ExternalInput").ap(),
        "KT": nc.dram_tensor("KT", [D, S], BF16, kind="ExternalInput").ap(),
        "VT": nc.dram_tensor("VT", [D, S], BF16, kind="ExternalInput").ap(),
        "WqT": nc.dram_tensor("WqT", [D, 256], BF16, kind="ExternalInput").ap(),
        "WkT": nc.dram_tensor("WkT", [D, 256], BF16, kind="ExternalInput").ap(),
        "WvT": nc.dram_tensor("WvT", [D, 256], BF16, kind="ExternalInput").ap(),
        "WoT": nc.dram_tensor("WoT", [256, D], BF16, kind="ExternalInput").ap(),
        "bqc": nc.dram_tensor("bqc", [128, NPAIRS], F32,
                              kind="ExternalInput").ap(),
        "bkc": nc.dram_tensor("bkc", [128, NPAIRS], F32,
                              kind="ExternalInput").ap(),
        "out": nc.dram_tensor("out", [S, D], BF16, kind="ExternalOutput").ap(),
    }
    with tile.TileContext(nc) as tc:
        _emit(tc, io)
    nc.compile()
    _CACHED = (nc, io)
    return _CACHED


def make_in_maps(Q, K, V, Wq, bq, Wk, bk, Wv, bv, Wo):
    """Build the 8 per-core input dicts (host-side sharding)."""
    Q = np.asarray(Q, np.float32)
    K = np.asarray(K, np.float32)
    V = np.asarray(V, np.float32)
    qt = [np.ascontiguousarray(Q[b].T).astype(NPBF16) for b in range(B)]
    kt = [np.ascontiguousarray(K[b].T).astype(NPBF16) for b in range(B)]
    vt = [np.ascontiguousarray(V[b].T).astype(NPBF16) for b in range(B)]
    in_maps = []
    for core in range(NCORES):
        b, g = divmod(core, 4)
        rows = slice(g * 256, (g + 1) * 256)
        in_maps.append({
            "QT": qt[b], "KT": kt[b], "VT": vt[b],
            "WqT": np.ascontiguousarray(
                np.asarray(Wq, np.float32)[rows].T).astype(NPBF16),
            "WkT": np.ascontiguousarray(
                np.asarray(Wk, np.float32)[rows].T).astype(NPBF16),
            "WvT": np.ascontiguousarray(
                np.asarray(Wv, np.float32)[rows].T).astype(NPBF16),
            "WoT": np.ascontiguousarray(
                np.asarray(Wo, np.float32)[:, rows].T).astype(NPBF16),
            "bqc": np.ascontiguousarray(
                np.asarray(bq, np.float32)[rows].reshape(2, 128).T),
            "bkc": np.ascontiguousarray(
                np.asarray(bk, np.float32)[rows].reshape(2, 128).T),
        })
    return in_maps


def kernel(Q, K, V, mask, Wq, bq, Wk, bk, Wv, bv, Wo, bo, _results_hook=None):
    nc, _io = _build()
    in_maps = make_in_maps(Q, K, V, Wq, bq, Wk, bk, Wv, bv, Wo)
    res = run_bass_kernel_spmd(nc, in_maps, core_ids=list(range(NCORES)))
    if _results_hook is not None:
        _results_hook(res)
    out = np.zeros((B, S, D), np.float32)
    for core in range(NCORES):
        out[core // 4] += np.asarray(res.results[core]["out"], np.float32)
    # bv passes through the softmax average exactly; its output-space
    # contribution is the constant row bv @ Wo.T, folded in here.
    out += np.asarray(bo, np.float32) + (
        np.asarray(bv, np.float32) @ np.asarray(Wo, np.float32).T)
    return out


# revision 40
# speedup vs baseline: 1.0530x; 1.0297x over previous
"""Self-contained Trainium2 Bass kernel for MultiHeadAttention (v3, bf16).

Problem: B=2, S=2048, D=1024, H=16, hd=64, with the reference's
masked_fill(mask==0, -1e-09) quirk: masked scores become ~0.0, so
exp(masked) == 1.0 in fp32 and every key position participates in the
softmax denominator. Fully-masked key blocks therefore contribute a
block-constant suffix sum of V rows (fs), added via cheap matmuls
instead of full score/attn matmuls.

All matmul numerics are bf16: fp8 anywhere in the value path costs
2-4.5% relative output error (softmax averaging shrinks the signal as
fast as the noise), beyond the 2e-2 gate.

v3 structural changes vs the original baseline:
  * Input DMA consolidated to ~16 large 3D-AP descriptors (was ~77),
    issued on sync/gpsimd so ScalarE keeps its cycles for exp.
  * v2 PSUM->SBUF copies batched: all 4 heads in one strided copy.
  * fs suffix adds batched across the 4 heads (15 TT ops, was 60).
  * Diagonal-mask affine_selects batched across the hl pair (32, was 64).
  * outproj emits one [128, 2, 512] PSUM tile per seq block: 4 matmuls,
    one DVE copy, one output DMA (was 2 copies + 2 DMAs).

Layouts (per core, all matmul operands at partition base 0):
  qt  [128, pair, S]   q^T, two heads stacked on partitions (d dims)
  ktz [128, head, S]   k^T zero-padded: even heads live on partitions
                       0-63 (64-127 zero), odd heads on 64-127 — the
                       scores matmul is then a plain K=128 matmul
                       against the pair-stacked qt.
  v2  [128, head, kj, 65]  V blocks with an appended ones column
                       (produces the softmax denominator for free).
  scores^T [sk, sq] in PSUM -> exp on ScalarE -> bf16 tiles ->
  attnU^T [65, sq] accumulated with V2 stationary, so no transposes
  are needed before the O-projection; rowsum = row 64.

Sharding: 8 cores = 2 batches x 4 head-groups (4 heads per core).
Each core computes a partial [S, D] output; the host sums the 4
partials per batch and adds bo.
"""

import numpy as np
import ml_dtypes

import concourse.bass as bass
import concourse.bacc as bacc
import concourse.tile as tile
import concourse.mybir as mybir
from concourse.bass_utils import run_bass_kernel_spmd

BF16 = mybir.dt.bfloat16
F32 = mybir.dt.float32
NPBF16 = ml_dtypes.bfloat16
AF = mybir.ActivationFunctionType

B = 2
S = 2048
D = 1024
H = 16
HD = 64
NCORES = 8
HPC = 4            # heads per core
NPAIRS = 2         # head pairs per core
NQ = S // 128      # 16 query/key blocks of 128
QCH = 512          # sq chunk width
NCH = S // QCH     # 4 chunks
KT = D // 128      # 8 contraction tiles for projections


def _emit(tc: tile.TileContext, io: dict):
    nc = tc.nc

    persist = tc.alloc_tile_pool(name="persist", bufs=1)

    # ---- constants ----
    ones128 = persist.tile([128, 128], BF16, name="ones128")
    nc.gpsimd.memset(ones128, 1.0)

    # ---- persistent SBUF arrays ----
    qt = persist.tile([128, NPAIRS, S], BF16, name="qt")
    ktz = persist.tile([128, HPC, S], BF16, name="ktz")
    v2 = persist.tile([128, HPC, NQ, 128], BF16, name="v2")
    fs = persist.tile([128, HPC, NQ, 128], BF16, name="fs")
    att = persist.tile([128, NPAIRS, S], BF16, name="att")

    qts = persist.tile([128, KT, S], BF16, name="qts")
    kts = persist.tile([128, KT, S], BF16, name="kts")
    vts = persist.tile([128, KT, S], BF16, name="vts")
    wqt = persist.tile([128, KT, 256], BF16, name="wqt")
    wkt = persist.tile([128, KT, 256], BF16, name="wkt")
    wvt = persist.tile([128, KT, 256], BF16, name="wvt")
    wot = persist.tile([128, NPAIRS, D], BF16, name="wot")
    # q/k biases as per-partition columns: bqc[:, p] = bq[p*128:(p+1)*128]
    bqc = persist.tile([128, NPAIRS], F32, name="bqc")
    bkc = persist.tile([128, NPAIRS], F32, name="bkc")

    # ---- input DMA: few large 3D-AP descriptors ----
    # gpsimd (SWDGE): small weight tensors, early; keeps ScalarE free
    nc.gpsimd.dma_start(wqt, io["WqT"].rearrange("(t p) m -> p t m", p=128))
    nc.gpsimd.dma_start(wkt, io["WkT"].rearrange("(t p) m -> p t m", p=128))
    nc.gpsimd.dma_start(wvt, io["WvT"].rearrange("(t p) m -> p t m", p=128))
    nc.gpsimd.dma_start(wot, io["WoT"].rearrange("(o p) m -> p o m", p=128))
    nc.scalar.dma_start(bqc, io["bqc"])
    nc.scalar.dma_start(bkc, io["bkc"])

    # sync (HWDGE): activations; chunk 0 first so projections start early
    def xt_src(t_io, c0, c1):
        return t_io.rearrange("(t p) s -> p t s", p=128)[:, :, c0:c1]

    dma = nc.sync
    # Lean first burst: ONLY the q/k chunk-0 slices.  The 16 SDMA
    # engines round-robin across all outstanding DMAs, so anything else
    # issued now would steal bandwidth from the critical-path bytes.
    # Later bursts are gated (dependency edges onto the first q/k
    # projection matmuls) so their descriptors enter the ring later.
    dma.dma_start(qts[:, 0:4, 0:QCH], xt_src(io["QT"], 0, QCH)[:, 0:4, :])
    dma.dma_start(qts[:, 4:8, 0:QCH], xt_src(io["QT"], 0, QCH)[:, 4:8, :])
    dma.dma_start(kts[:, 0:4, 0:QCH], xt_src(io["KT"], 0, QCH)[:, 0:4, :])
    dma.dma_start(kts[:, 4:8, 0:QCH], xt_src(io["KT"], 0, QCH)[:, 4:8, :])
    burst2 = [
        dma.dma_start(vts[:, :, 0:QCH], xt_src(io["VT"], 0, QCH)),
        dma.dma_start(vts[:, :, QCH:2 * QCH], xt_src(io["VT"], QCH, 2 * QCH)),
        dma.dma_start(vts[:, :, 2 * QCH:], xt_src(io["VT"], 2 * QCH, S)),
    ]
    burst3 = [
        dma.dma_start(qts[:, :, QCH:2 * QCH], xt_src(io["QT"], QCH, 2 * QCH)),
        dma.dma_start(kts[:, :, QCH:2 * QCH], xt_src(io["KT"], QCH, 2 * QCH)),
        dma.dma_start(qts[:, :, 2 * QCH:], xt_src(io["QT"], 2 * QCH, S)),
        dma.dma_start(kts[:, :, 2 * QCH:], xt_src(io["KT"], 2 * QCH, S)),
    ]
    stage_gates = {"burst2": burst2, "burst3": burst3}

    nc.gpsimd.memset(v2[:, :, :, 64:128], 1.0)  # 64 ones columns -> Z on rows 64-127

    pb_s = tc.alloc_tile_pool(name="pb_scores", bufs=2, space="PSUM")
    pb_a = tc.alloc_tile_pool(name="pb_attnu", bufs=2, space="PSUM")
    pb_e = tc.alloc_tile_pool(name="pb_exp", bufs=9)
    pb_o = tc.alloc_tile_pool(name="pb_ob", bufs=2)
    pb_r = tc.alloc_tile_pool(name="pb_recip", bufs=2)

    def vproj_unit(st):
        if True:
            psv_t = pb_s.tile([128, 2, QCH], F32, tag="sps", name=f"ps_v{st}")
            ps_v = psv_t[:, 0, 0:256]
            for t in range(KT):
                nc.tensor.matmul(ps_v, vts[:, t, st * 128:(st + 1) * 128],
                                 wvt[:, t, :], start=(t == 0),
                                 stop=(t == KT - 1))
            # bv is NOT added here: it passes through the softmax average
            # exactly (weights sum to 1), so the host folds bv @ Wo.T into
            # the bo add instead.
            # one strided copy: all 4 heads at once
            nc.vector.tensor_copy(v2[:, :, st, 0:64],
                                  ps_v.rearrange("p (h d) -> p h d", h=4))

    def qproj_unit(c, p):
        sq = slice(c * QCH, (c + 1) * QCH)
        psq_t = pb_s.tile([128, 2, QCH], F32, tag="sps", name=f"ps_q{p}_{c}")
        ps_q = psq_t[:, 0, :]
        for t in range(KT):
            mm = nc.tensor.matmul(ps_q, wqt[:, t, p * 128:(p + 1) * 128],
                                  qts[:, t, sq], start=(t == 0),
                                  stop=(t == KT - 1))
            if c == 0 and p == 0 and t == 0 and "burst2" in stage_gates:
                for dd in stage_gates.pop("burst2"):
                    tile.add_dep_helper(dd.ins, mm.ins, reason="stage b2")
        # bias folded into the PSUM->SBUF copy (per-partition scalar add)
        nc.vector.tensor_scalar_add(qt[:, p, sq], ps_q, bqc[:, p:p + 1])

    def kproj_unit(c, p):
        sq = slice(c * QCH, (c + 1) * QCH)
        psk_t = pb_s.tile([128, 2, QCH], F32, tag="sps", name=f"ps_k{p}_{c}")
        ps_k = psk_t[:, 0, :]
        for t in range(KT):
            mm = nc.tensor.matmul(ps_k, wkt[:, t, p * 128:(p + 1) * 128],
                                  kts[:, t, sq], start=(t == 0),
                                  stop=(t == KT - 1))
            if c == 0 and p == 0 and t == 0 and "burst3" in stage_gates:
                for dd in stage_gates.pop("burst3"):
                    tile.add_dep_helper(dd.ins, mm.ins, reason="stage b3")
        nc.vector.tensor_scalar_add(ktz[0:64, 2 * p, sq], ps_k[0:64, :],
                                    bkc[0:64, p:p + 1])
        nc.vector.tensor_scalar_add(ktz[64:128, 2 * p + 1, sq],
                                    ps_k[64:128, :], bkc[64:128, p:p + 1])

    def qkproj(c):
        for p in range(NPAIRS):
            qproj_unit(c, p)
            kproj_unit(c, p)

    def folded_suffixes():
        nc.vector.memset(fs[:, :, NQ - 1, :], 0.0)
        for q in range(NQ - 2, -1, -1):
            # all 4 heads in one strided TT add
            nc.vector.tensor_add(fs[:, :, q, :], fs[:, :, q + 1, :],
                                 v2[:, :, q + 1, :])

    aups_tiles = {}

    def chunk_loop(c, fillers=()):
        """scores -> exp -> attnU^T accumulation for chunk c, both pairs.

        Software-pipelined LEAD score units ahead of attnU (the PE queue
        is strict FIFO, so attnU's exp-wait would otherwise serialize
        every kj).  One filler work-unit (an independent PE job: v/q/k
        projection unit, outproj unit, fs chain step) is emitted per kj
        iteration so ScalarE's exp stream never starves while pure-PE
        phases run.
        """
        fillers = list(fillers)
        nkj = 4 * c + 4
        exts = {}
        LEAD = 3

        def scores_unit(p, kj):
            c0 = max(kj - 4 * c, 0) * 128   # first valid col in chunk
            sps = pb_s.tile([128, 2, QCH], F32, tag="sps",
                            name=f"sps{p}_{c}_{kj}")
            for hl in range(2):
                # K=64 on row-group hl*64: the two heads' score matmuls
                # occupy disjoint 64-row strips of the PE array and run
                # concurrently (tile_position derived from the operand
                # partition bases)
                h64 = slice(hl * 64, hl * 64 + 64)
                nc.tensor.matmul(
                    sps[:, hl, c0:QCH],
                    ktz[h64, 2 * p + hl, kj * 128:(kj + 1) * 128],
                    qt[h64, p, c * QCH + c0:(c + 1) * QCH],
                    start=True, stop=True)
            ext = pb_e.tile([128, 2, QCH], BF16, tag="ext",
                            name=f"ext{p}_{c}_{kj}")
            nc.scalar.activation(ext[:, :, c0:QCH], sps[:, :, c0:QCH],
                                 AF.Exp, scale=0.125)
            if kj >= 4 * c:  # diagonal block: masked exp entries -> 1.0
                nc.gpsimd.affine_select(
                    out=ext[:, :, c0:c0 + 128],
                    in_=ext[:, :, c0:c0 + 128],
                    compare_op=mybir.AluOpType.is_ge,
                    fill=1.0, base=0,
                    pattern=[[0, 2], [1, 128]], channel_multiplier=-1)
            exts[(p, kj)] = ext

        def attnu_unit(p, kj, aups):
            c0 = max(kj - 4 * c, 0) * 128
            ext = exts.pop((p, kj))
            for hl in range(2):
                # masked cols < c0 get their (block-constant)
                # contribution from the FS matmuls
                nc.tensor.matmul(
                    aups[:, hl, c0:QCH],
                    v2[:, 2 * p + hl, kj, :],
                    ext[:, hl, c0:QCH],
                    start=(kj == 0),
                    stop=(kj == nkj - 1 and c > 0))

        aups_t = {}
        for p in range(NPAIRS):
            aups_t[p] = pb_a.tile([128, 2, QCH], F32, tag="aups",
                                  name=f"aups{p}_{c}")
            aups_tiles[(p, c)] = aups_t[p]
            scores_unit(p, 0)
        # pairs interleaved: two independent score->exp->attnU chains keep
        # ScalarE fed when either chain waits on a cross-engine hazard
        for kj in range(nkj):
            if kj + 1 < nkj:
                for p in range(NPAIRS):
                    scores_unit(p, kj + 1)
            for p in range(NPAIRS):
                attnu_unit(p, kj, aups_t[p])
                if c > 0 and 1 <= kj <= 4:
                    # fs suffix adds, spread one per kj, in the order the
                    # fs chain produces them (high qi first); they commute
                    # with the accumulation
                    ql = 4 - kj
                    qi = 4 * c + ql
                    if qi < NQ - 1:
                        for hl in range(2):
                            nc.tensor.matmul(
                                aups_t[p][:, hl, ql * 128:(ql + 1) * 128],
                                fs[:, 2 * p + hl, qi, :], ones128,
                                start=False, stop=False)
                slots_left = (nkj - 1 - kj) * NPAIRS + (NPAIRS - 1 - p)
                want = len(fillers) - slots_left  # drain evenly
                for _ in range(max(1 if fillers else 0, want)):
                    if fillers:
                        fillers.pop(0)()
        if c > 0:
            # finalize both pairs now: hides the recip chain inside the
            # chunk instead of stalling the next chunk's attnU on the
            # aups buffer WAR
            for p in range(NPAIRS):
                finalize_pair(p, c)
        for f in fillers:
            f()

    def finalize_pair(p, c):
        """Rowsum reciprocal + normalize into att for (pair, chunk)."""
        ch = slice(c * QCH, (c + 1) * QCH)
        aups = aups_tiles[(p, c)]
        lnr = pb_r.tile([128, 2 * QCH], F32, tag="lr", name=f"lnr{p}_{c}")
        nc.scalar.activation(lnr[64:128, :], aups[64:128, :, :], AF.Ln)
        nc.scalar.activation(lnr[64:128, :], lnr[64:128, :], AF.Exp,
                             scale=-1.0)  # in-place: lnr becomes 1/Z
        for hl in range(2):
            nc.vector.tensor_mul(
                att[hl * 64:(hl + 1) * 64, p, ch],
                aups[0:64, hl, :],
                lnr[64:128, hl * QCH:(hl + 1) * QCH])

    def finalize0():
        """Chunk 0 only: late FS adds, then per-pair finalize."""
        for p in range(NPAIRS):
            aups = aups_tiles[(p, 0)]
            for hl in range(2):
                for ql in range(4):
                    nc.tensor.matmul(
                        aups[:, hl, ql * 128:(ql + 1) * 128],
                        fs[:, 2 * p + hl, ql, :], ones128,
                        start=False, stop=(ql == 3))
            finalize_pair(p, 0)

    def outproj_unit(st):
        pso = pb_s.tile([128, 2, QCH], F32, tag="sps", name=f"pso{st}")
        for dc in range(2):
            for p in range(NPAIRS):
                nc.tensor.matmul(
                    pso[:, dc, :],
                    att[:, p, st * 128:(st + 1) * 128],
                    wot[:, p, dc * 512:(dc + 1) * 512],
                    start=(p == 0), stop=(p == NPAIRS - 1))
        ob = pb_o.tile([128, 2, QCH], BF16, tag="ob", name=f"ob{st}")
        nc.vector.tensor_copy(ob, pso)
        dma.dma_start(
            io["out"][st * 128:(st + 1) * 128, :].rearrange(
                "s (a m) -> s a m", a=2), ob)

    def vproj_filler(st):
        """One V-projection block + the fs chain step it unblocks."""
        def f():
            vproj_unit(st)
            q = st - 1
            if q >= 4:
                fs_step(q)
            elif st == 4:
                for qq in range(3, -1, -1):
                    fs_step(qq)
        return f

    def fs_step(q):
        # all 4 heads in one strided TT add
        nc.vector.tensor_add(fs[:, :, q, :], fs[:, :, q + 1, :],
                             v2[:, :, q + 1, :])
    # Emission order == per-engine FIFO order (the scheduler follows
    # priorities).  Chunk 0's fillers run the remaining V projections in
    # DESCENDING block order with the fs suffix-chain steps interleaved
    # (the chain consumes blocks high-to-low), then the chunk-1 q/k
    # projections, so finalize0 is ready the moment chunk 0 drains.
    nc.vector.memset(fs[:, :, NQ - 1, :], 0.0)
    # HAM warm-up: ~24 dependency-free matmuls on the ones tile run while
    # the first input DMA streams, so real matmuls start at 2.4 GHz
    warm = pb_s.tile([128, 2, QCH], F32, tag="sps", name="warm")
    for _ in range(24):
        nc.tensor.matmul(warm[:, 0, 0:128], ones128, ones128,
                         start=True, stop=True)
    qkproj(0)
    for st in range(4):
        vproj_unit(st)
    c0_fillers = [vproj_filler(st) for st in range(15, 3, -1)]
    c0_fillers += [lambda: qproj_unit(1, 0), lambda: kproj_unit(1, 0),
                   lambda: qproj_unit(1, 1), lambda: kproj_unit(1, 1)]
    chunk_loop(0, c0_fillers)
    finalize0()
    c1_fillers = [lambda p=p: qproj_unit(2, p) for p in range(2)]
    c1_fillers += [lambda p=p: kproj_unit(2, p) for p in range(2)]
    c1_fillers += [lambda st=st: outproj_unit(st) for st in range(0, 4)]
    chunk_loop(1, c1_fillers)
    c2_fillers = [lambda p=p: qproj_unit(3, p) for p in range(2)]
    c2_fillers += [lambda p=p: kproj_unit(3, p) for p in range(2)]
    c2_fillers += [lambda st=st: outproj_unit(st) for st in range(4, 8)]
    chunk_loop(2, c2_fillers)
    c3_fillers = [lambda st=st: outproj_unit(st) for st in range(8, 12)]
    chunk_loop(3, c3_fillers)
    for st in range(12, 16):
        outproj_unit(st)

    pb_r.release()
    pb_o.release()
    pb_e.release()
    pb_a.release()
    pb_s.release()
    persist.release()


_CACHED = None


def _patch_act_tables():
    """Make Exp and Ln resolve to the single combined table set so the
    per-chunk recip (Ln/Exp) doesn't thrash ACT_TABLE_LOADs against the
    softmax Exp calls."""
    from concourse import hw_specs
    orig = hw_specs.get_activation_tables

    def patched(arch):
        t = dict(orig(arch))
        if "natural_log_exp_and_others" in t:
            for name in t:
                if name != "natural_log_exp_and_others":
                    t[name] = t[name] - {AF.Exp, AF.Ln}
        return t

    bacc.get_activation_tables = patched


def _build():
    global _CACHED
    if _CACHED is not None:
        return _CACHED
    _patch_act_tables()
    nc = bacc.Bacc("TRN2", target_bir_lowering=False, debug=False)
    io = {
        "QT": nc.dram_tensor("QT", [D, S], BF16, kind="ExternalInput").ap(),
        "KT": nc.dram_tensor("KT", [D, S], BF16, kind="ExternalInput").ap(),
        "VT": nc.dram_tensor("VT", [D, S], BF16, kind="ExternalInput").ap(),
        "WqT": nc.dram_tensor("WqT", [D, 256], BF16, kind="ExternalInput").ap(),
        "WkT": nc.dram_tensor("WkT", [D, 256], BF16, kind="ExternalInput").ap(),
        "WvT": nc.dram_tensor("WvT", [D, 256], BF16, kind="ExternalInput").ap(),
        "WoT": nc.dram_tensor("WoT", [256, D], BF16, kind="ExternalInput").ap(),
        "bqc": nc.dram_tensor("bqc", [128, NPAIRS], F32,
                              kind="ExternalInput").ap(),
        "bkc": nc.dram_tensor("bkc", [128, NPAIRS], F32,
                              kind="ExternalInput").ap(),
        "out": nc.dram_tensor("out", [S, D], BF16, kind="ExternalOutput").ap(),
    }
    with tile.TileContext(nc) as tc:
        _emit(tc, io)
    nc.compile()
    _CACHED = (nc, io)
    return _CACHED


def make_in_maps(Q, K, V, Wq, bq, Wk, bk, Wv, bv, Wo):
    """Build the 8 per-core input dicts (host-side sharding)."""
    Q = np.asarray(Q, np.float32)
    K = np.asarray(K, np.float32)
    V = np.asarray(V, np.float32)
    qt = [np.ascontiguousarray(Q[b].T).astype(NPBF16) for b in range(B)]
    kt = [np.ascontiguousarray(K[b].T).astype(NPBF16) for b in range(B)]
    vt = [np.ascontiguousarray(V[b].T).astype(NPBF16) for b in range(B)]
    in_maps = []
    for core in range(NCORES):
        b, g = divmod(core, 4)
        rows = slice(g * 256, (g + 1) * 256)
        in_maps.append({
            "QT": qt[b], "KT": kt[b], "VT": vt[b],
            "WqT": np.ascontiguousarray(
                np.asarray(Wq, np.float32)[rows].T).astype(NPBF16),
            "WkT": np.ascontiguousarray(
                np.asarray(Wk, np.float32)[rows].T).astype(NPBF16),
            "WvT": np.ascontiguousarray(
                np.asarray(Wv, np.float32)[rows].T).astype(NPBF16),
            "WoT": np.ascontiguousarray(
                np.asarray(Wo, np.float32)[:, rows].T).astype(NPBF16),
            "bqc": np.ascontiguousarray(
                np.asarray(bq, np.float32)[rows].reshape(2, 128).T),
            "bkc": np.ascontiguousarray(
                np.asarray(bk, np.float32)[rows].reshape(2, 128).T),
        })
    return in_maps


def kernel(Q, K, V, mask, Wq, bq, Wk, bk, Wv, bv, Wo, bo, _results_hook=None):
    nc, _io = _build()
    in_maps = make_in_maps(Q, K, V, Wq, bq, Wk, bk, Wv, bv, Wo)
    res = run_bass_kernel_spmd(nc, in_maps, core_ids=list(range(NCORES)))
    if _results_hook is not None:
        _results_hook(res)
    out = np.zeros((B, S, D), np.float32)
    for core in range(NCORES):
        out[core // 4] += np.asarray(res.results[core]["out"], np.float32)
    # bv passes through the softmax average exactly; its output-space
    # contribution is the constant row bv @ Wo.T, folded in here.
    out += np.asarray(bo, np.float32) + (
        np.asarray(bv, np.float32) @ np.asarray(Wo, np.float32).T)
    return out
